# revision 11
# baseline (speedup 1.0000x reference)
"""Causal self-attention (B=2, T=2048, C=768, H=12) on 8 TRN2 NeuronCores.

Sharding: core i handles batch b = i//4 and 3 consecutive heads h0 = 3*(i%4).
Each core produces a partial projection output [T, C] (sum over its 3 heads);
the host sums the 4 partials per batch and adds biases.

Per-core dataflow (all transposeless):
  - QK gen:  psum[128,512] = sum_ct Wqk[ct,h].T @ xT[ct]  -> rows 0:64 = Q^T
             (scale+bias folded), rows 64:128 = K^T.
  - V gen:   psum[128,192] = sum_ct xT[ct,tchunk].T @ Wv[ct] -> v in natural
             [T, hs] layout, stored per k-tile as [v | 1] (ones col -> denom).
  - Attn:    S^T tile = K_block @ Q^T  ([128 kpos, 512 q] in PSUM), exp on ACT
             (no max subtraction; scores are O(1)), causal mask multiply on
             diagonal tiles only, PV accumulates [y^T | denom] over k-tiles.
  - Norm:    recip(denom) broadcast across partitions via a K=1 matmul,
             y^T = y_unnorm^T * bcast.
  - Proj:    out[tchunk, :] = sum_h yT[h, tchunk].T @ Wp[h]  (PSUM -> DRAM).
"""

import os

os.environ.setdefault("MYCRO_LOCAL_CACHE", "1")

import numpy as np

import concourse.bass as bass
import concourse.bacc as bacc
import concourse.mybir as mybir
import concourse.tile as tile
from concourse.bass_utils import run_bass_kernel_spmd

T = 2048
C = 768
HS = 64
NH = 12
HPC = 3  # heads per core
NCORES = 8
CT = C // 128  # 6 contraction tiles for qkv/v gen
QC = 512  # q-chunk width
NQC = T // QC  # 4
NKT = T // 128  # 16 k-tiles (and T-chunks)
SCALE = 1.0 / 8.0  # 1/sqrt(HS)
F32 = mybir.dt.float32

_PROGRAM = None


def _build_program():
    nc = bacc.Bacc("TRN2")
    xT_d = nc.declare_dram_parameter("xT", [128, CT, T], F32, isOutput=False)
    wqk_d = nc.declare_dram_parameter("wqk", [128, CT, HPC, 128], F32, isOutput=False)
    wv_d = nc.declare_dram_parameter("wv", [128, CT, HPC * HS], F32, isOutput=False)
    wp_d = nc.declare_dram_parameter("wp", [HS, HPC, C], F32, isOutput=False)
    bq_d = nc.declare_dram_parameter("bq", [HS, HPC], F32, isOutput=False)
    mask_d = nc.declare_dram_parameter("mask", [128, QC], F32, isOutput=False)
    out_d = nc.declare_dram_parameter("out", [T, C], F32, isOutput=True)

    with tile.TileContext(nc) as tc:
        with (
            tc.tile_pool(name="const", bufs=1) as constp,
            tc.tile_pool(name="big", bufs=1) as bigp,
            tc.tile_pool(name="exps", bufs=4) as expp,
            tc.tile_pool(name="work", bufs=3) as workp,
            tc.tile_pool(name="ps_s", bufs=2, space="PSUM") as ps_s,
            tc.tile_pool(name="ps_y", bufs=2, space="PSUM") as ps_y,
            tc.tile_pool(name="ps_m", bufs=2, space="PSUM") as ps_m,
        ):
            # ---- inputs -> SBUF
            xT = bigp.tile([128, CT, T], F32)
            nc.sync.dma_start(out=xT, in_=xT_d[:])
            wqk = constp.tile([128, CT, HPC, 128], F32)
            nc.sync.dma_start(out=wqk, in_=wqk_d[:])
            wv = constp.tile([128, CT, HPC * HS], F32)
            nc.sync.dma_start(out=wv, in_=wv_d[:])
            wp = constp.tile([HS, HPC, C], F32)
            nc.sync.dma_start(out=wp, in_=wp_d[:])
            bq = constp.tile([HS, HPC], F32)
            nc.sync.dma_start(out=bq, in_=bq_d[:])
            mask = constp.tile([128, QC], F32)
            nc.sync.dma_start(out=mask, in_=mask_d[:])
            ones = constp.tile([128, HS], F32)
            nc.vector.memset(ones, 1.0)

            qT = bigp.tile([HS, HPC, T], F32)
            kT = bigp.tile([HS, HPC, T], F32)
            vsb = bigp.tile([128, NKT, HPC, HS + 1], F32)  # [v | 1] per head
            yT = bigp.tile([HS, HPC, T], F32)

            nc.vector.memset(vsb[:, :, :, HS], 1.0)

            # ---- QK^T generation (head 0 first so attention can start early)
            def qkgen(h):
                for jq in range(NQC):
                    pqk = ps_m.tile([128, QC], F32, tag="misc")
                    for ct in range(CT):
                        nc.tensor.matmul(
                            pqk,
                            wqk[:, ct, h, :],
                            xT[:, ct, jq * QC : (jq + 1) * QC],
                            start=(ct == 0),
                            stop=(ct == CT - 1),
                        )
                    nc.vector.tensor_scalar_add(
                        qT[:, h, jq * QC : (jq + 1) * QC],
                        pqk[0:HS, :],
                        bq[:, h : h + 1],
                    )
                    # partition-shifting evacuation (64:128 -> 0:64); legal on
                    # DVE at 64 channels (bank0->Q0, bank1->Q1, reads follow
                    # the src access pattern)
                    nc.vector.tensor_copy(
                        kT[:, h, jq * QC : (jq + 1) * QC], pqk[64:128, :]
                    )

            qkgen(0)

            # ---- V generation (natural [T, hs] layout, + ones column)
            for m in range(NKT):
                pv = ps_m.tile([128, QC], F32, tag="misc")
                for ct in range(CT):
                    nc.tensor.matmul(
                        pv[:, 0 : HPC * HS],
                        xT[:, ct, m * 128 : (m + 1) * 128],
                        wv[:, ct, :],
                        start=(ct == 0),
                        stop=(ct == CT - 1),
                    )
                nc.vector.tensor_copy(
                    vsb[:, m, :, 0:HS],
                    pv[:, 0 : HPC * HS].rearrange("p (h d) -> p h d", h=HPC),
                )

            qkgen(1)
            qkgen(2)

            # ---- attention + normalize + projection, per q-chunk
            for jq in range(NQC):
                q0 = jq * QC
                for h in range(HPC):
                    py = ps_y.tile([128, QC], F32, tag="py")
                    # group g covers k-tiles (2g, 2g+1); last two groups are
                    # the narrowed diagonal tiles r=0..3 (m = 4*jq + r).
                    ngroups = 2 * jq + 2
                    es_tiles = []
                    for g in range(ngroups):
                        es_p = ps_s.tile([128, 2 * QC], F32, tag="es")
                        es_b = expp.tile([128, 2 * QC], F32, tag="ex")
                        es_tiles.append((es_p, es_b))
                        for s in range(2):
                            kt = 2 * g + s
                            if kt < 4 * jq:  # full tile
                                r = None
                                w = QC
                                qoff = 0
                            else:
                                r = kt - 4 * jq
                                w = QC - 128 * r
                                qoff = 128 * r
                            nc.tensor.matmul(
                                es_p[:, s * QC : s * QC + w],
                                kT[:, h, kt * 128 : (kt + 1) * 128],
                                qT[:, h, q0 + qoff : q0 + QC],
                                start=True,
                                stop=True,
                            )
                    # exp (ACT), mask-mul diagonal tiles (DVE)
                    for g in range(ngroups):
                        es_p, es_b = es_tiles[g]
                        kt0 = 2 * g
                        if kt0 + 1 < 4 * jq:  # both full
                            nc.scalar.activation(
                                es_b[:, 0 : 2 * QC],
                                es_p[:, 0 : 2 * QC],
                                mybir.ActivationFunctionType.Exp,
                            )
                        else:
                            r0 = kt0 - 4 * jq  # 0 or 2
                            if r0 == 0:  # widths 512, 384: contiguous span
                                nc.scalar.activation(
                                    es_b[:, 0 : QC + 384],
                                    es_p[:, 0 : QC + 384],
                                    mybir.ActivationFunctionType.Exp,
                                )
                            else:  # widths 256, 128: two disjoint spans
                                nc.scalar.activation(
                                    es_b[:, 0:256],
                                    es_p[:, 0:256],
                                    mybir.ActivationFunctionType.Exp,
                                )
                                nc.scalar.activation(
                                    es_b[:, QC : QC + 128],
                                    es_p[:, QC : QC + 128],
                                    mybir.ActivationFunctionType.Exp,
                                )
                            for s in range(2):
                                r = kt0 + s - 4 * jq
                                w = QC - 128 * r
                                nc.vector.tensor_mul(
                                    es_b[:, s * QC : s * QC + w],
                                    es_b[:, s * QC : s * QC + w],
                                    mask[:, 0:w],
                                )
                    # PV accumulation
                    for g in range(ngroups):
                        _, es_b = es_tiles[g]
                        for s in range(2):
                            kt = 2 * g + s
                            if kt < 4 * jq:
                                w = QC
                                qoff = 0
                            else:
                                r = kt - 4 * jq
                                w = QC - 128 * r
                                qoff = 128 * r
                            nc.tensor.matmul(
                                py[0 : HS + 1, qoff:QC],
                                vsb[:, kt, h, :],
                                es_b[:, s * QC : s * QC + w],
                                start=(kt == 0),
                                stop=(kt == 4 * jq + 3),
                                skip_group_check=True,
                            )
                    # normalize: yT = py[0:64] * bcast(1/denom)
                    rc = workp.tile([128, QC], F32, tag="rc")
                    nc.vector.reciprocal(rc[64:65, :], py[64:65, :])
                    pb = ps_m.tile([128, QC], F32, tag="misc")
                    nc.tensor.matmul(
                        pb[0:HS, :],
                        ones[64:65, 0:HS],
                        rc[64:65, :],
                        start=True,
                        stop=True,
                    )
                    bc = workp.tile([128, QC], F32, tag="bc")
                    nc.vector.tensor_copy(bc[0:HS, :], pb[0:HS, :])
                    nc.vector.tensor_mul(
                        yT[:, h, q0 : q0 + QC], py[0:HS, :], bc[0:HS, :]
                    )
                # projection for this q-chunk's 4 T-chunks
                for t in range(4 * jq, 4 * jq + 4):
                    ob = workp.tile([128, C], F32, tag="ob")
                    for n0, w in ((0, 512), (512, 256)):
                        po = ps_m.tile([128, QC], F32, tag="misc")
                        for h in range(HPC):
                            nc.tensor.matmul(
                                po[:, 0:w],
                                yT[:, h, t * 128 : (t + 1) * 128],
                                wp[:, h, n0 : n0 + w],
                                start=(h == 0),
                                stop=(h == HPC - 1),
                            )
                        nc.vector.tensor_copy(ob[:, n0 : n0 + w], po[:, 0:w])
                    nc.sync.dma_start(
                        out=out_d[t * 128 : (t + 1) * 128, :], in_=ob
                    )
    return nc


def get_program():
    global _PROGRAM
    if _PROGRAM is None:
        _PROGRAM = _build_program()
        if not _PROGRAM.is_finalized():
            _PROGRAM.finalize()
    return _PROGRAM


def make_in_maps(x, W_attn, b_attn):
    x = np.asarray(x, dtype=np.float32)
    W_attn = np.asarray(W_attn, dtype=np.float32)
    b_attn = np.asarray(b_attn, dtype=np.float32)
    mask_arr = (
        np.arange(128, dtype=np.int64)[:, None] <= np.arange(QC, dtype=np.int64)[None, :]
    ).astype(np.float32)
    in_maps = []
    for i in range(NCORES):
        b = i // 4
        h0 = HPC * (i % 4)
        xb = x[b]  # [T, C]
        xT_arr = np.ascontiguousarray(
            xb.T.reshape(CT, 128, T).transpose(1, 0, 2)
        )  # [p, ct, t]
        Wq = (
            W_attn[:, h0 * HS : (h0 + HPC) * HS].reshape(C, HPC, HS) * SCALE
        )
        Wk = W_attn[:, C + h0 * HS : C + (h0 + HPC) * HS].reshape(C, HPC, HS)
        wqk_full = np.concatenate([Wq, Wk], axis=2)  # [C, HPC, 128]
        wqk_arr = np.ascontiguousarray(
            wqk_full.reshape(CT, 128, HPC, 128).transpose(1, 0, 2, 3)
        )
        wv_arr = np.ascontiguousarray(
            W_attn[:, 2 * C + h0 * HS : 2 * C + (h0 + HPC) * HS]
            .reshape(CT, 128, HPC * HS)
            .transpose(1, 0, 2)
        )
        bq_arr = np.ascontiguousarray(
            (b_attn[h0 * HS : (h0 + HPC) * HS] * SCALE).reshape(HPC, HS).T
        )
        in_maps.append(
            {
                "xT": xT_arr,
                "wqk": wqk_arr,
                "wv": wv_arr,
                "bq": bq_arr,
                "mask": mask_arr,
            }
        )
    return in_maps


def add_wp(in_maps, W_proj):
    W_proj = np.asarray(W_proj, dtype=np.float32)
    for i in range(NCORES):
        h0 = HPC * (i % 4)
        in_maps[i]["wp"] = np.ascontiguousarray(
            W_proj[h0 * HS : (h0 + HPC) * HS, :].reshape(HPC, HS, C).transpose(1, 0, 2)
        )
    return in_maps


def gather(results, b_attn, W_proj, b_proj):
    b_attn = np.asarray(b_attn, dtype=np.float32)
    W_proj = np.asarray(W_proj, dtype=np.float32)
    b_proj = np.asarray(b_proj, dtype=np.float32)
    parts = [np.asarray(r["out"], dtype=np.float32) for r in results]
    out = np.stack(
        [parts[0] + parts[1] + parts[2] + parts[3], parts[4] + parts[5] + parts[6] + parts[7]]
    )
    # b_v adds to y after normalization -> constant vector through the proj.
    # b_k provably cancels in softmax; b_q is handled on-device.
    const = b_proj + b_attn[2 * C : 3 * C] @ W_proj
    return out + const[None, None, :]


def run(x, W_attn, b_attn, W_proj, b_proj, trace=False):
    nc = get_program()
    in_maps = add_wp(make_in_maps(x, W_attn, b_attn), W_proj)
    res = run_bass_kernel_spmd(nc, in_maps, list(range(NCORES)), trace=trace)
    out = gather(res.results, b_attn, W_proj, b_proj)
    return out, res


def kernel(x, W_attn, b_attn, W_proj, b_proj):
    out, _ = run(x, W_attn, b_attn, W_proj, b_proj, trace=False)
    return out
